# revision 30
# baseline (speedup 1.0000x reference)
"""NetVLAD kernel for Trainium2, data-parallel over N across 8 NeuronCores.

Math (per sample; reference semantics):
    xn   = x / ||x||_2(channels)                  # per-pixel L2 norm, eps never binds
    u    = (conv_w @ xn)                          # [K, S]; conv_b cancels in softmax over S
    a    = softmax_S(u)                           # => sum_S a == 1 exactly
    agg  = a @ xn^T                               # [K, C]
    vlad = agg - centroids                        # since a_sum == 1
    out  = l2norm_flat(l2norm_rows(vlad))         # row norms -> 1, so flat norm == 8

Implementation notes:
  - Everything streamed through the PE in bf16; fp32 output error is ~1e-5
    because vlad is dominated by the fp32-exact centroids and all
    normalizations cancel common-mode scale error.
  - x loaded via SWDGE cast-DMA (fp32 HBM -> bf16 SBUF), transposed to [S, C]
    tiles with the XBAR DMA transpose (no compute-engine cost).
  - sum_c x^2 per pixel via DVE scalar_tensor_tensor fused accum_out.
  - logits computed transposed ([s, k]) so the softmax scale q and exp fuse
    into one ACT pass with per-partition scale.
  - agg accumulated for 2 samples into one PSUM bank (output base-partition
    0/64), fixups run on full 128-partition tiles.
"""

import numpy as np

import concourse.bacc as bacc
import concourse.bass as bass
import concourse.tile as tile
from concourse import mybir

f32 = mybir.dt.float32
bf16 = mybir.dt.bfloat16
Alu = mybir.AluOpType
Act = mybir.ActivationFunctionType

N, C, H, W, K = 64, 512, 32, 32, 64
S = H * W            # 1024
NCORES = 8
NPC = N // NCORES    # samples per core
CT = C // 128        # 4 channel tiles
ST = S // 128        # 8 spatial tiles


def _patch_act_tables():
    """Make the greedy ACT table-set chooser land every function we use
    (Exp, Ln, Square) on `natural_log_exp_and_others`, so the whole kernel
    needs exactly ONE table load. Without this, Exp resolves to set 0 and Ln
    to set 5, and the loader thrashes (~1.3us per switch, dozens per run).
    Set ids stay aligned with act_info.json (only set *contents* are
    filtered), so codegen remains valid."""
    import concourse.hw_specs as hs

    if getattr(hs, "_netvlad_patched", False):
        return
    orig = hs.get_activation_tables

    def patched(arch):
        t = orig(arch)
        keep = t.get("natural_log_exp_and_others", set())
        return {
            name: (fns if name == "natural_log_exp_and_others" else fns - keep)
            for name, fns in t.items()
        }

    hs.get_activation_tables = patched
    hs._netvlad_patched = True
    bacc.get_activation_tables = patched


def build_program(reps: int = 1) -> bass.Bass:
    _patch_act_tables()
    nc = bacc.Bacc("TRN2", target_bir_lowering=False, debug=False)

    x_in = nc.declare_dram_parameter("x", [NPC, C, H, W], f32, isOutput=False)
    w_in = nc.declare_dram_parameter("conv_w", [K, C], f32, isOutput=False)
    cent_in = nc.declare_dram_parameter("centroids", [K, C], f32, isOutput=False)
    y_out = nc.declare_dram_parameter("y", [NPC, K * C], f32, isOutput=True)

    with tile.TileContext(nc) as tc:
        with (
            tc.tile_pool(name="const", bufs=1) as const_pool,
            tc.tile_pool(name="xb", bufs=4) as xb_pool,
            tc.tile_pool(name="xbt", bufs=4) as xbt_pool,
            tc.tile_pool(name="ework", bufs=4) as e_pool,
            tc.tile_pool(name="stats", bufs=6) as stat_pool,
            tc.tile_pool(name="scratch", bufs=8) as scratch_pool,
            tc.tile_pool(name="vout", bufs=3) as v_pool,
            tc.tile_pool(name="ps_raw", bufs=2, space="PSUM") as ps_raw,
            tc.tile_pool(name="ps_agg", bufs=2, space="PSUM") as ps_agg,
            tc.tile_pool(name="ps_z", bufs=2, space="PSUM") as ps_z,
        ):
            # ---- constants ----

            # conv_w -> bf16, then wT tiles [128c, 64k] per c-tile
            wb_nat = const_pool.tile([64, C], bf16)
            nc.gpsimd.dma_start(out=wb_nat, in_=w_in[:, :])  # cast f32->bf16
            # one-shot transpose [64, 512] -> [512, 64] laid out [p, ct, k]
            # (middle output dim = high bits of the logical row index)
            wTb = const_pool.tile([128, CT, K], bf16)
            nc.sync.dma_start(out=wTb, in_=wb_nat, transpose=True)

            # centroids stacked twice along partitions (2 samples per group)
            # keep plain copies on SWDGE (gpsimd): the HWDGE xbar serializes
            # on every transpose<->copy mode switch, so HWDGE carries ONLY
            # transposes
            cent2 = const_pool.tile([128, C], f32)
            nc.gpsimd.dma_start(out=cent2[0:64, :], in_=cent_in[:, :])
            nc.gpsimd.dma_start(out=cent2[64:128, :], in_=cent_in[:, :])

            import contextlib

            loop_ctx = (
                tc.For_i(0, reps, 1, name="reps")
                if reps > 1
                else contextlib.nullcontext()
            )
            with loop_ctx:
                _emit_samples(nc, tc, x_in, y_out, wTb, cent2,
                              xb_pool, xbt_pool, e_pool, stat_pool,
                              scratch_pool, v_pool, ps_raw, ps_agg, ps_z)
    nc.compile()
    return nc


def _emit_samples(nc, tc, x_in, y_out, wTb, cent2, xb_pool, xbt_pool, e_pool,
                  stat_pool, scratch_pool, v_pool, ps_raw, ps_agg, ps_z):
    if True:
        if True:
            aggps = None
            zps = None
            for n in range(NPC):
                pa = 64 * (n % 2)

                # ---- load x[n] as bf16, [128, CT, S]; two half-loads so the
                # first transposes can start at half-load ----
                xb = xb_pool.tile([128, CT, S], bf16)
                src = x_in[n].rearrange("(t p) h w -> p t (h w)", p=128)
                nc.gpsimd.dma_start(out=xb, in_=src)

                # ---- logits^T matmuls emitted early (need only xb) ----
                rawT = ps_raw.tile([128, ST * K], f32)  # st-slices of 64
                for st in range(ST):
                    for ct in range(CT):
                        nc.tensor.matmul(
                            out=rawT[:, st * K:(st + 1) * K],
                            lhsT=xb[:, ct, st * 128:(st + 1) * 128],
                            rhs=wTb[:, ct, :],
                            start=(ct == 0),
                            stop=(ct == CT - 1),
                        )

                # ---- ONE whole-sample transpose via XBAR (SP ring only:
                # concurrent transposes from both HWDGE rings race on the
                # single xbar unit and corrupt results). Layout
                # [s, ct, st, c]: logical row j=ct*1024+st*128+s lands at
                # dense free offset (ct*8+st)*128 + c. ----
                xbT = xbt_pool.tile([128, CT, ST, 128], bf16)
                nc.sync.dma_start(out=xbT, in_=xb, transpose=True)

                # ---- per-pixel 1/||x||: ss = sum_c x^2 (fused accum) ----
                # split the square+reduce across DVE and ACT by tile
                ss8 = stat_pool.tile([128, ST], f32)
                for st in range(ST):
                    sq_scr = scratch_pool.tile([128, C], bf16, tag="sq_scr")
                    if st < 7:
                        nc.vector.scalar_tensor_tensor(
                            out=sq_scr,
                            in0=xbT[:, :, st, :],
                            scalar=1.0,
                            in1=xbT[:, :, st, :],
                            op0=Alu.mult,
                            op1=Alu.mult,
                            accum_out=ss8[:, st:st + 1],
                        )
                    else:
                        nc.scalar.activation(
                            out=sq_scr,
                            in_=xbT[:, :, st, :],
                            func=Act.Square,
                            accum_out=ss8[:, st:st + 1],
                        )
                # q = ss^-1/2 = exp(-0.5*ln(ss)); ln/exp live in ONE ACT
                # table set (natural_log_exp_and_others) -> no table thrash.
                # Processed in two half-tiles so exps for st 0..3 can start
                # before the second half of the sum-of-squares finishes.
                lnss = stat_pool.tile([128, ST], f32)
                q8 = stat_pool.tile([128, ST], f32)
                lnq8 = stat_pool.tile([128, ST], f32)
                nrmb = stat_pool.tile([128, ST], bf16)
                h = ST // 2
                for lo, hi in ((0, h), (h, ST)):
                    nc.scalar.activation(
                        out=lnss[:, lo:hi], in_=ss8[:, lo:hi], func=Act.Ln
                    )
                    nc.scalar.activation(
                        out=q8[:, lo:hi], in_=lnss[:, lo:hi], func=Act.Exp,
                        scale=-0.5,
                    )
                    # lnq = -0.5*ln(ss) (bias for the fused exp below)
                    nc.vector.tensor_scalar_mul(
                        lnq8[:, lo:hi], lnss[:, lo:hi], -0.5
                    )
                    # nrm = ss^+1/2 as bf16 column vector (Z matmul rhs)
                    nc.scalar.activation(
                        out=nrmb[:, lo:hi], in_=lnss[:, lo:hi], func=Act.Exp,
                        scale=0.5,
                    )

                # ---- G^T = q*exp(q*rawT) = exp(q*rawT + ln q), fused in ACT ----
                GbT = e_pool.tile([128, ST * K], bf16)
                for st in range(ST):
                    nc.scalar.activation(
                        out=GbT[:, st * K:(st + 1) * K],
                        in_=rawT[:, st * K:(st + 1) * K],
                        func=Act.Exp,
                        scale=q8[:, st:st + 1],
                        bias=lnq8[:, st:st + 1],
                    )

                # ---- agg[k, c] += G^T.T @ xT ; Z[k] += G^T.T @ nrm = sum E ----
                if pa == 0:
                    aggps = ps_agg.tile([128, C], f32)
                    zps = ps_z.tile([128, 1], f32)
                for st in range(ST):
                    nc.tensor.matmul(
                        out=aggps[pa:pa + 64, :],
                        lhsT=GbT[:, st * K:(st + 1) * K],
                        rhs=xbT[:, :, st, :],
                        start=(st == 0),
                        stop=(st == ST - 1),
                    )
                    nc.tensor.matmul(
                        out=zps[pa:pa + 64, :],
                        lhsT=GbT[:, st * K:(st + 1) * K],
                        rhs=nrmb[:, st:st + 1],
                        start=(st == 0),
                        stop=(st == ST - 1),
                    )

                # ---- fixups for the 2-sample group ----
                if pa == 64:
                    vr = stat_pool.tile([128, 1], f32)
                    nc.vector.reciprocal(out=vr, in_=zps)
                    vlad = v_pool.tile([128, C], f32)
                    # vlad = agg * (1/Z) - centroids
                    nc.vector.scalar_tensor_tensor(
                        out=vlad,
                        in0=aggps,
                        scalar=vr,
                        in1=cent2,
                        op0=Alu.mult,
                        op1=Alu.subtract,
                    )
                    # row sumsq -> scale = 1/(8*sqrt(rs)) = 1/sqrt(64*rs)
                    rs = stat_pool.tile([128, 1], f32)
                    sq_scr2 = scratch_pool.tile([128, C], bf16, tag="sq_scr")
                    nc.scalar.activation(
                        out=sq_scr2, in_=vlad, func=Act.Square, accum_out=rs
                    )
                    # scale = 1/sqrt(64*rs) = exp(-0.5*ln(64*rs))
                    lnrs = stat_pool.tile([128, 1], f32)
                    nc.scalar.activation(out=lnrs, in_=rs, func=Act.Ln, scale=64.0)
                    sc = stat_pool.tile([128, 1], f32)
                    nc.scalar.activation(out=sc, in_=lnrs, func=Act.Exp, scale=-0.5)
                    outb = v_pool.tile([128, C], f32)
                    nc.vector.tensor_scalar_mul(outb, vlad, sc)
                    # output on SP HWDGE: putting it on Pool would delay the
                    # next sample's cast-DMA issue (Pool SEQ is serial)
                    nc.sync.dma_start(
                        out=y_out[n - 1:n + 1].rearrange("a (p f) -> (a p) f", p=64),
                        in_=outb,
                    )


_prog_cache: dict[str, bass.Bass] = {}


def _get_program() -> bass.Bass:
    if "nc" not in _prog_cache:
        _prog_cache["nc"] = build_program()
    return _prog_cache["nc"]


def kernel(x, conv_w, conv_b, centroids, trace=False):
    from concourse.bass_utils import run_bass_kernel_spmd

    x = np.ascontiguousarray(np.asarray(x, dtype=np.float32))
    conv_w = np.ascontiguousarray(np.asarray(conv_w, dtype=np.float32))
    centroids = np.ascontiguousarray(np.asarray(centroids, dtype=np.float32))

    nc = _get_program()
    in_maps = [
        {
            "x": x[i * NPC:(i + 1) * NPC],
            "conv_w": conv_w,
            "centroids": centroids,
        }
        for i in range(NCORES)
    ]
    res = run_bass_kernel_spmd(nc, in_maps, list(range(NCORES)), trace=trace)
    out = np.concatenate([res.results[i]["y"] for i in range(NCORES)], axis=0)
    if trace:
        return out, res
    return out
